# revision 2
# baseline (speedup 1.0000x reference)
"""Trainium2 Bass kernel v2 for nn_DiGCN (2-layer GCN + TimeEncode), 8 cores.

Key restructuring vs v1:
- b1=b2=0 => h2 = A2(A2 h W1)W2 = (A2^2 h)(W1@W2): the dense transforms
  commute out. Device does two gather/segment-sum passes + one final 128x128
  matmul per window. No phase A, no per-layer W matmuls, no dinv scaling
  (dinv[src]*dinv[dst] folded into the per-edge one-hot weights on host).
- h = x + cos(t*freq + phase) precomputed on host as the layer-1 gather
  table (bf16, node-major, padded block layout) - an ExternalInput.
- Block node ownership: core c owns padded rows [c*12544, (c+1)*12544);
  ONE AllGather of the layer-1 result (bf16) builds the layer-2 table.
- Gather config tunable (elem_rows over-read trick for 512B descriptors,
  src-sorted streams) per microbenchmark results.
"""
import math
import numpy as np

import sys
if "/opt/trn_rl_repo" not in sys.path:
    sys.path.insert(0, "/opt/trn_rl_repo")

from contextlib import ExitStack

import concourse.bass as bass
import concourse.tile as tile
from concourse.bass_types import AP
from concourse import bacc, mybir
from concourse.bass_utils import run_bass_kernel_spmd
from concourse import library_config
from concourse.masks import make_identity

P = 128
NCORES = 8
D = 128
N_NODES = 100000
REAL_PER_CORE = 12500
SPAD = 12544                     # per-core padded block (98 windows)
NPAD = SPAD * NCORES             # 100352
BUCKET = 25088                   # gather bucket rows (int16-addressable)
NB = NPAD // BUCKET              # 4
NWIN = SPAD // P                 # 98

# gather config (tuned via gather_bench: unsorted 256B rows from private
# tables hit ~211 GB/s/core; run-sorted and over-read variants were slower)
CFG = {
    "elem_rows": 1,      # table rows per descriptor
    "gcall": 8192,       # indices per gather call
    "sort": False,       # (b,w)-run src sort benched SLOWER than unsorted
    "single_packet": False,
    "msg_bufs": 3,
    "tab2_space": "Local",
}


class Plan:
    pass


def wrap_idx(idx):
    """[n] -> [128, n/16] int16: idx wrapped into 16 partitions, tiled 8x."""
    n = len(idx)
    a = idx.reshape(n // 16, 16).T
    return np.ascontiguousarray(np.tile(a, (8, 1))).astype(np.int16)


def build_plan(src_gpos, dst_gpos, wgt_e, gcall):
    pl = Plan()
    e_sb = (src_gpos // BUCKET).astype(np.int32)
    e_idx = (src_gpos % BUCKET).astype(np.int32)
    e_core = (dst_gpos // SPAD).astype(np.int32)
    dpos = dst_gpos % SPAD
    e_w = (dpos // P).astype(np.int32)
    e_dloc = (dpos % P).astype(np.int32)

    counts = np.zeros((NCORES, NB, NWIN), dtype=np.int64)
    np.add.at(counts, (e_core, e_sb, e_w), 1)
    K = np.ceil(counts / P).astype(np.int64).max(axis=0)    # [NB, NWIN]
    assert (K.sum(axis=0) > 0).all(), "window with no chunks"
    pl.K = K
    pl.nchunk = int(K.sum())
    pl.border = list(range(NB))

    # schedule: (bucket, window) repeated K times, bucket-major
    sched = []
    for b in pl.border:
        for w in range(NWIN):
            sched.extend([(b, w)] * int(K[b, w]))
    pl.sched = sched

    # run start offsets (chunk index of first chunk of each (b,w) run)
    starts = {}
    acc = 0
    for b in pl.border:
        for w in range(NWIN):
            starts[(b, w)] = acc
            acc += int(K[b, w])
    pl.run_start = starts

    # last bucket (in border order) with chunks for each window
    lastb = {}
    for b in pl.border:
        for w in range(NWIN):
            if K[b, w] > 0:
                lastb[w] = b
    pl.last_bucket = lastb

    # gather calls: per bucket, chunks split into calls of <= gcall/P chunks
    pl.gcall = gcall
    calls = []
    pos = 0
    for b in pl.border:
        cb = int(K[b].sum())
        s = 0
        while s < cb:
            c = min(gcall // P, cb - s)
            calls.append((b, pos + s, c))
            s += c
        pos += cb
    pl.calls = calls

    # per-core streams in schedule order
    if CFG["sort"]:
        order = np.lexsort((e_idx, e_w, e_sb, e_core))
    else:
        order = np.lexsort((e_w, e_sb, e_core))
    osrc, ow, odloc, owgt, ocore, osb = (
        e_idx[order], e_w[order], e_dloc[order], wgt_e[order],
        e_core[order], e_sb[order])

    nch = pl.nchunk
    # pad slots get RANDOM in-bucket indices (dloc=-1 zeroes their weight):
    # constant-0 pads would make ~12% of all descriptors hit one 256B row —
    # an HBM bank hotspot shared by all 8 cores
    rng = np.random.default_rng(12345)
    idx_s = rng.integers(0, BUCKET - 1, (NCORES, nch * P)).astype(np.int16)
    dloc_s = np.full((NCORES, nch * P), -1.0, dtype=np.float32)
    wgt_s = np.zeros((NCORES, nch * P), dtype=np.float32)

    chunk_off = np.zeros((NB, NWIN), dtype=np.int64)
    for (b, w), st in starts.items():
        chunk_off[b, w] = st

    ptr = np.searchsorted(ocore, np.arange(NCORES + 1))
    for c in range(NCORES):
        lo, hi = ptr[c], ptr[c + 1]
        csb, cw = osb[lo:hi], ow[lo:hi]
        keys = csb.astype(np.int64) * NWIN + cw
        uq, inv, cnts = np.unique(keys, return_inverse=True, return_counts=True)
        grp_start = np.concatenate([[0], np.cumsum(cnts)])[:-1]
        local = np.arange(hi - lo) - grp_start[inv]
        gpos = (chunk_off[csb, cw] * P + local).astype(np.int64)
        idx_s[c, gpos] = osrc[lo:hi].astype(np.int16)
        dloc_s[c, gpos] = odloc[lo:hi].astype(np.float32)
        wgt_s[c, gpos] = owgt[lo:hi]

    pl.idx_stream = idx_s
    pl.dloc_stream = dloc_s
    pl.wgt_stream = wgt_s
    return pl


def preprocess(x, edge_index, t_index, W1, W2, freq,
               phase=None, b1=None, b2=None, gcall=None):
    n = x.shape[0]
    assert n == N_NODES
    gcall = gcall or CFG["gcall"]
    x = np.asarray(x, np.float32)
    t_index = np.asarray(t_index, np.float32)
    freq = np.asarray(freq, np.float32)
    phase_a = np.zeros(D, np.float32) if phase is None else np.asarray(phase, np.float32)

    src = np.asarray(edge_index[0], dtype=np.int64)
    dst = np.asarray(edge_index[1], dtype=np.int64)
    loop = np.arange(n, dtype=np.int64)
    src = np.concatenate([src, loop])
    dst = np.concatenate([dst, loop])
    deg = np.bincount(dst, minlength=n).astype(np.float64)
    dinv = np.where(deg > 0, 1.0 / np.sqrt(np.maximum(deg, 1e-12)), 0.0)
    dinv = dinv.astype(np.float32)

    # node -> padded global position
    nid = np.arange(n, dtype=np.int64)
    core = nid // REAL_PER_CORE
    gpos = core * SPAD + (nid - core * REAL_PER_CORE)

    wgt_e = (dinv[src] * dinv[dst]).astype(np.float32)
    pl = build_plan(gpos[src], gpos[dst], wgt_e, gcall)

    # layer-1 table: h = x + cos(t freq + phase), padded block layout (+pad
    # rows for the multi-row over-read)
    h = x + np.cos(t_index[:, None] * freq[None, :] + phase_a[None, :])
    import ml_dtypes
    tab1 = np.zeros((NPAD + 2, D), dtype=ml_dtypes.bfloat16)
    tab1[gpos] = h.astype(ml_dtypes.bfloat16)

    W12 = (np.asarray(W1, np.float64) @ np.asarray(W2, np.float64)).astype(np.float32)

    nch = pl.nchunk

    # host-built one-hot eq stream, partition-major [P(edge), nch, P(dst)]
    # bf16: eqs[e, g, dloc[e,g]] = wgt[e,g]. Streamed per call via HWDGE
    # (contiguous 16KB/partition reads) — per-chunk DVE tensor_scalar builds
    # would lock SWDGE out of its descriptor rings (2-port perf mode).
    eqs = np.zeros((NCORES, P, nch, P), dtype=ml_dtypes.bfloat16)
    dl_all = pl.dloc_stream.reshape(NCORES, nch, P)        # [c, g, e]
    wg_all = pl.wgt_stream.reshape(NCORES, nch, P)
    for c in range(NCORES):
        g_i, e_i = np.nonzero(dl_all[c] >= 0)
        eqs[c, e_i, g_i, dl_all[c, g_i, e_i].astype(np.int64)] = \
            wg_all[c, g_i, e_i].astype(ml_dtypes.bfloat16)

    mx = max(c for _, _, c in pl.calls) * P
    gi = np.zeros((NCORES, len(pl.calls), P, mx // 16), dtype=np.int16)
    for ci, (b, s0, c) in enumerate(pl.calls):
        seg = pl.idx_stream[:, s0 * P:(s0 + c) * P]
        for cc in range(NCORES):
            w = wrap_idx(seg[cc])
            gi[cc, ci, :, :w.shape[1]] = w

    in_maps = []
    for c in range(NCORES):
        in_maps.append({
            "tab1": tab1,
            "gidx": np.ascontiguousarray(gi[c]),
            "eqs": np.ascontiguousarray(eqs[c]),
            "W12": W12,
        })

    # host-side correction for nonzero b1/b2 (zero in this problem):
    # out += s[:,None]*(b1@W2)[None,:] + b2, s[n] = sum_{e->n} dinv_s*dinv_d
    corr = None
    if (b1 is not None and np.any(b1)) or (b2 is not None and np.any(b2)):
        s = np.zeros(n, np.float64)
        np.add.at(s, dst, wgt_e.astype(np.float64))
        corr = np.zeros((n, D), np.float32)
        if b1 is not None and np.any(b1):
            corr += s[:, None].astype(np.float32) * (np.asarray(b1, np.float64)
                                                     @ np.asarray(W2, np.float64)
                                                     ).astype(np.float32)[None, :]
        if b2 is not None and np.any(b2):
            corr += np.asarray(b2, np.float32)[None, :]
    return pl, in_maps, gpos, corr


def build_program(pl, reps=1):
    import os as _os
    DEBUG_L1 = _os.environ.get("K2_DEBUG_L1") == "1"
    DEBUG_L2 = _os.environ.get("K2_DEBUG_L2") == "1"
    NOMM = _os.environ.get("K2_NOMM") == "1"        # timing ablation only
    CONSTEQ = _os.environ.get("K2_CONSTEQ") == "1"  # timing ablation only
    bf16, f32, i16 = mybir.dt.bfloat16, mybir.dt.float32, mybir.dt.int16
    nch = pl.nchunk
    mxcall = max(c for _, _, c in pl.calls)
    er = CFG["elem_rows"]
    esize = er * D
    sp = CFG["single_packet"]

    nc = bacc.Bacc("TRN2", target_bir_lowering=False, debug=False,
                   num_devices=NCORES, num_swdge_queues=4)
    tab1_t = nc.dram_tensor("tab1", [NPAD + 2, D], bf16, kind="ExternalInput")
    gidx = nc.dram_tensor("gidx", [len(pl.calls), P, (mxcall * P) // 16], i16,
                          kind="ExternalInput").ap()
    eqs = nc.dram_tensor("eqs", [P, nch, P], bf16, kind="ExternalInput").ap()
    W12 = nc.dram_tensor("W12", [D, D], f32, kind="ExternalInput").ap()
    out = nc.dram_tensor("out", [SPAD, D], f32, kind="ExternalOutput").ap()
    dbg_acc = None
    if DEBUG_L1:
        dbg_acc = nc.dram_tensor("dbg_acc", [P, NWIN * P], f32,
                                 kind="ExternalOutput").ap()
    dbg_tab2 = None
    if DEBUG_L2:
        dbg_tab2 = nc.dram_tensor("dbg_tab2", [NPAD, D], bf16,
                                  kind="ExternalOutput").ap()

    ag_in = nc.dram_tensor("ag_in", [SPAD, D], bf16)
    tab2_t = nc.dram_tensor("tab2", [NPAD, D], bf16,
                            addr_space=CFG["tab2_space"])

    def tab_view(tensor, b, nrows_cap):
        """Bucket-b gather view with er-row elements (over-read trick)."""
        nrows = min(BUCKET, nrows_cap - b * BUCKET - (er - 1))
        if er == 1:
            return tensor.ap()[b * BUCKET: b * BUCKET + nrows, :]
        return AP(tensor, b * BUCKET * D, [[D, nrows], [1, esize]])

    with tile.TileContext(nc) as tc, ExitStack() as ctx:
        const = ctx.enter_context(tc.tile_pool(name="const", bufs=1))
        accp = ctx.enter_context(tc.tile_pool(name="accp", bufs=1))
        sbM = ctx.enter_context(tc.tile_pool(name="sbM", bufs=CFG["msg_bufs"]))
        sbE = ctx.enter_context(tc.tile_pool(name="sbE", bufs=3))
        sbO = ctx.enter_context(tc.tile_pool(name="sbO", bufs=4))
        ipP = ctx.enter_context(tc.tile_pool(name="ipP", bufs=3))
        psR = ctx.enter_context(tc.tile_pool(name="psR", bufs=4, space="PSUM"))
        psM = ctx.enter_context(tc.tile_pool(name="psM", bufs=2, space="PSUM"))

        nc.gpsimd.load_library(library_config.mlp)

        ident = const.tile([P, P], f32)
        make_identity(nc, ident[:])
        w12t = const.tile([P, P], f32, tag="w12t")
        nc.sync.dma_start(w12t[:], W12[:])
        zerot = const.tile([P, P], f32, tag="zerot")
        nc.vector.memset(zerot[:], 0.0)

        acc = accp.tile([P, NWIN * P], f32)

        for _rep in range(reps):
            nc.vector.memset(acc[:], 0.0)

            def tt_copy(dst_ap, src_ap):
                # tensor_tensor never enters DVE 2-port perf mode (which
                # would lock SWDGE out of its descriptor rings); plain
                # tensor_copy/tensor_scalar would stall concurrent gathers
                nc.vector.tensor_tensor(dst_ap, src_ap, zerot[:],
                                        mybir.AluOpType.add)

            def edge_pass(tab_tensor, nrows_cap, layer):
                qrr = [0]
                chunk_pos = 0
                open_run = {}
                drained = set()
                for ci, (b, s0, ncall) in enumerate(pl.calls):
                    it = ipP.tile([P, (mxcall * P) // 16], i16, tag="idx")
                    nc.sync.dma_start(it[:], gidx[ci])
                    msg = sbM.tile([P, mxcall, esize], bf16, tag="msg")
                    nc.gpsimd.dma_gather(
                        msg[:, :ncall, :], tab_view(tab_tensor, b, nrows_cap),
                        it[:, :(ncall * P) // 16],
                        ncall * P, ncall * P, esize,
                        elem_step=(D if er > 1 else None),
                        single_packet=sp, queue_num=qrr[0] % 4)
                    qrr[0] += 1
                    et = sbE.tile([P, mxcall, P], bf16, tag="eq")
                    if not CONSTEQ:
                        # ACT's HWDGE ring: parallel to sync's (idx/cast/out)
                        nc.scalar.dma_start(et[:, :ncall, :],
                                            eqs[:, s0:s0 + ncall, :])
                    if NOMM:
                        chunk_pos += ncall
                        continue
                    for j in range(ncall):
                        g = chunk_pos + j
                        bb, ww = pl.sched[g]
                        kk = int(pl.K[bb, ww])
                        off = g - pl.run_start[(bb, ww)]
                        if off == 0:
                            open_run[ww] = psR.tile([P, P], f32, tag="run",
                                                    name="runps")
                        ps = open_run[ww]
                        nc.tensor.matmul(ps[:], lhsT=msg[:, j, :D],
                                         rhs=et[:, j, :],
                                         start=(off == 0), stop=(off == kk - 1))
                        if off == kk - 1:
                            aw = acc[:, ww * P:(ww + 1) * P]
                            if ww in drained:
                                nc.vector.tensor_add(aw, aw, ps[:])
                            else:
                                tt_copy(aw, ps[:])
                                drained.add(ww)
                            del open_run[ww]
                            if pl.last_bucket[ww] == bb:
                                if layer == 1:
                                    # table rows must be node-major: transpose
                                    # the [feat, dst] acc window via PE
                                    tps = psM.tile([P, P], f32, tag="tps")
                                    nc.tensor.transpose(out=tps[:], in_=aw,
                                                        identity=ident[:])
                                    xws = sbO.tile([P, P], bf16, tag="xws")
                                    tt_copy(xws[:], tps[:])
                                    nc.sync.dma_start(
                                        ag_in.ap()[ww * P:(ww + 1) * P, :],
                                        xws[:])
                                else:
                                    ops = psM.tile([P, D], f32, tag="ops")
                                    nc.tensor.matmul(ops[:], lhsT=aw,
                                                     rhs=w12t[:],
                                                     start=True, stop=True)
                                    ot = sbO.tile([P, D], f32, tag="ot")
                                    tt_copy(ot[:], ops[:])
                                    nc.sync.dma_start(
                                        out[ww * P:(ww + 1) * P, :], ot[:])
                    chunk_pos += ncall

            edge_pass(tab1_t, NPAD + 2, 1)

            if DEBUG_L1:
                nc.sync.dma_start(dbg_acc[:], acc[:])
                continue

            nc.gpsimd.collective_compute(
                "AllGather", mybir.AluOpType.bypass,
                ins=[ag_in.ap()[:]], outs=[tab2_t.ap()[:]],
                replica_groups=[list(range(NCORES))])

            if DEBUG_L2:
                for bb in range(NB):
                    nc.sync.dma_start(
                        dbg_tab2[bb * BUCKET:(bb + 1) * BUCKET, :],
                        tab2_t.ap()[bb * BUCKET:(bb + 1) * BUCKET, :])
                continue

            # layer-2 drains overwrite acc (copy-first per window); Tile WAR
            # tracking orders them after the layer-1 casts
            edge_pass(tab2_t, NPAD, 2)

    nc.compile()
    return nc


_PROG_CACHE = {}


def run(x, edge_index, t_index, W1, W2, freq, phase=None, b1=None, b2=None,
        nc_prog=None):
    pl, in_maps, gpos, corr = preprocess(x, edge_index, t_index, W1, W2, freq,
                                         phase, b1, b2)
    if nc_prog is not None:
        nc = nc_prog
    else:
        key = (pl.nchunk, len(pl.calls), CFG["elem_rows"], CFG["gcall"])
        if key not in _PROG_CACHE:
            _PROG_CACHE[key] = build_program(pl)
        nc = _PROG_CACHE[key]
    res = run_bass_kernel_spmd(nc, in_maps, list(range(NCORES)))
    full = np.concatenate([res.results[c]["out"] for c in range(NCORES)], axis=0)
    out = full[gpos]
    if corr is not None:
        out = out + corr
    return np.ascontiguousarray(out.astype(np.float32))


def kernel(x, edge_index, t_index, W1, b1, W2, b2, freq, phase):
    x = np.asarray(x, dtype=np.float32)
    t_index = np.asarray(t_index, dtype=np.float32)
    return run(x, np.asarray(edge_index), t_index,
               np.asarray(W1, np.float32), np.asarray(W2, np.float32),
               np.asarray(freq, np.float32),
               phase=np.asarray(phase, np.float32),
               b1=np.asarray(b1, np.float32), b2=np.asarray(b2, np.float32))


# revision 3
# speedup vs baseline: 1.0435x; 1.0435x over previous
"""Trainium2 Bass kernel v2 for nn_DiGCN (2-layer GCN + TimeEncode), 8 cores.

Key restructuring vs v1:
- b1=b2=0 => h2 = A2(A2 h W1)W2 = (A2^2 h)(W1@W2): the dense transforms
  commute out. Device does two gather/segment-sum passes + one final 128x128
  matmul per window. No phase A, no per-layer W matmuls, no dinv scaling
  (dinv[src]*dinv[dst] folded into the per-edge one-hot weights on host).
- h = x + cos(t*freq + phase) precomputed on host as the layer-1 gather
  table (bf16, node-major, padded block layout) - an ExternalInput.
- Block node ownership: core c owns padded rows [c*12544, (c+1)*12544);
  ONE AllGather of the layer-1 result (bf16) builds the layer-2 table.
- Gather config tunable (elem_rows over-read trick for 512B descriptors,
  src-sorted streams) per microbenchmark results.
"""
import math
import numpy as np

import sys
if "/opt/trn_rl_repo" not in sys.path:
    sys.path.insert(0, "/opt/trn_rl_repo")

from contextlib import ExitStack

import concourse.bass as bass
import concourse.tile as tile
from concourse.bass_types import AP
from concourse import bacc, mybir
from concourse.bass_utils import run_bass_kernel_spmd
from concourse import library_config
from concourse.masks import make_identity

P = 128
NCORES = 8
D = 128
N_NODES = 100000
REAL_PER_CORE = 12500
SPAD = 12544                     # per-core padded block (98 windows)
NPAD = SPAD * NCORES             # 100352
BUCKET = 25088                   # gather bucket rows (int16-addressable)
NB = NPAD // BUCKET              # 4
NWIN = SPAD // P                 # 98

# gather config (tuned via gather_bench: unsorted 256B rows from private
# tables hit ~211 GB/s/core; run-sorted and over-read variants were slower)
CFG = {
    "elem_rows": 1,      # table rows per descriptor
    "gcall": 8192,       # indices per gather call
    "sort": False,       # (b,w)-run src sort benched SLOWER than unsorted
    "single_packet": False,
    "msg_bufs": 4,
    "tab2_space": "Local",
}


class Plan:
    pass


def wrap_idx(idx):
    """[n] -> [128, n/16] int16: idx wrapped into 16 partitions, tiled 8x."""
    n = len(idx)
    a = idx.reshape(n // 16, 16).T
    return np.ascontiguousarray(np.tile(a, (8, 1))).astype(np.int16)


def build_plan(src_gpos, dst_gpos, wgt_e, gcall):
    pl = Plan()
    e_sb = (src_gpos // BUCKET).astype(np.int32)
    e_idx = (src_gpos % BUCKET).astype(np.int32)
    e_core = (dst_gpos // SPAD).astype(np.int32)
    dpos = dst_gpos % SPAD
    e_w = (dpos // P).astype(np.int32)
    e_dloc = (dpos % P).astype(np.int32)

    counts = np.zeros((NCORES, NB, NWIN), dtype=np.int64)
    np.add.at(counts, (e_core, e_sb, e_w), 1)
    K = np.ceil(counts / P).astype(np.int64).max(axis=0)    # [NB, NWIN]
    assert (K.sum(axis=0) > 0).all(), "window with no chunks"
    pl.K = K
    pl.nchunk = int(K.sum())
    pl.border = list(range(NB))

    # schedule: (bucket, window) repeated K times, bucket-major
    sched = []
    for b in pl.border:
        for w in range(NWIN):
            sched.extend([(b, w)] * int(K[b, w]))
    pl.sched = sched

    # run start offsets (chunk index of first chunk of each (b,w) run)
    starts = {}
    acc = 0
    for b in pl.border:
        for w in range(NWIN):
            starts[(b, w)] = acc
            acc += int(K[b, w])
    pl.run_start = starts

    # last bucket (in border order) with chunks for each window
    lastb = {}
    for b in pl.border:
        for w in range(NWIN):
            if K[b, w] > 0:
                lastb[w] = b
    pl.last_bucket = lastb

    # gather calls: per bucket, chunks split into calls of <= gcall/P chunks
    pl.gcall = gcall
    calls = []
    pos = 0
    for b in pl.border:
        cb = int(K[b].sum())
        s = 0
        while s < cb:
            c = min(gcall // P, cb - s)
            calls.append((b, pos + s, c))
            s += c
        pos += cb
    pl.calls = calls

    # per-core streams in schedule order
    if CFG["sort"]:
        order = np.lexsort((e_idx, e_w, e_sb, e_core))
    else:
        order = np.lexsort((e_w, e_sb, e_core))
    osrc, ow, odloc, owgt, ocore, osb = (
        e_idx[order], e_w[order], e_dloc[order], wgt_e[order],
        e_core[order], e_sb[order])

    nch = pl.nchunk
    # pad slots get RANDOM in-bucket indices (dloc=-1 zeroes their weight):
    # constant-0 pads would make ~12% of all descriptors hit one 256B row —
    # an HBM bank hotspot shared by all 8 cores
    rng = np.random.default_rng(12345)
    idx_s = rng.integers(0, BUCKET - 1, (NCORES, nch * P)).astype(np.int16)
    dloc_s = np.full((NCORES, nch * P), -1.0, dtype=np.float32)
    wgt_s = np.zeros((NCORES, nch * P), dtype=np.float32)

    chunk_off = np.zeros((NB, NWIN), dtype=np.int64)
    for (b, w), st in starts.items():
        chunk_off[b, w] = st

    ptr = np.searchsorted(ocore, np.arange(NCORES + 1))
    for c in range(NCORES):
        lo, hi = ptr[c], ptr[c + 1]
        csb, cw = osb[lo:hi], ow[lo:hi]
        keys = csb.astype(np.int64) * NWIN + cw
        uq, inv, cnts = np.unique(keys, return_inverse=True, return_counts=True)
        grp_start = np.concatenate([[0], np.cumsum(cnts)])[:-1]
        local = np.arange(hi - lo) - grp_start[inv]
        gpos = (chunk_off[csb, cw] * P + local).astype(np.int64)
        idx_s[c, gpos] = osrc[lo:hi].astype(np.int16)
        dloc_s[c, gpos] = odloc[lo:hi].astype(np.float32)
        wgt_s[c, gpos] = owgt[lo:hi]

    pl.idx_stream = idx_s
    pl.dloc_stream = dloc_s
    pl.wgt_stream = wgt_s
    return pl


def preprocess(x, edge_index, t_index, W1, W2, freq,
               phase=None, b1=None, b2=None, gcall=None):
    n = x.shape[0]
    assert n == N_NODES
    gcall = gcall or CFG["gcall"]
    x = np.asarray(x, np.float32)
    t_index = np.asarray(t_index, np.float32)
    freq = np.asarray(freq, np.float32)
    phase_a = np.zeros(D, np.float32) if phase is None else np.asarray(phase, np.float32)

    src = np.asarray(edge_index[0], dtype=np.int64)
    dst = np.asarray(edge_index[1], dtype=np.int64)
    loop = np.arange(n, dtype=np.int64)
    src = np.concatenate([src, loop])
    dst = np.concatenate([dst, loop])
    deg = np.bincount(dst, minlength=n).astype(np.float64)
    dinv = np.where(deg > 0, 1.0 / np.sqrt(np.maximum(deg, 1e-12)), 0.0)
    dinv = dinv.astype(np.float32)

    # node -> padded global position
    nid = np.arange(n, dtype=np.int64)
    core = nid // REAL_PER_CORE
    gpos = core * SPAD + (nid - core * REAL_PER_CORE)

    wgt_e = (dinv[src] * dinv[dst]).astype(np.float32)
    pl = build_plan(gpos[src], gpos[dst], wgt_e, gcall)

    # layer-1 table: h = x + cos(t freq + phase), padded block layout (+pad
    # rows for the multi-row over-read)
    h = x + np.cos(t_index[:, None] * freq[None, :] + phase_a[None, :])
    import ml_dtypes
    tab1 = np.zeros((NPAD + 2, D), dtype=ml_dtypes.bfloat16)
    tab1[gpos] = h.astype(ml_dtypes.bfloat16)

    W12 = (np.asarray(W1, np.float64) @ np.asarray(W2, np.float64)).astype(np.float32)

    nch = pl.nchunk

    # host-built one-hot eq stream, partition-major [P(edge), nch, P(dst)]
    # bf16: eqs[e, g, dloc[e,g]] = wgt[e,g]. Streamed per call via HWDGE
    # (contiguous 16KB/partition reads) — per-chunk DVE tensor_scalar builds
    # would lock SWDGE out of its descriptor rings (2-port perf mode).
    eqs = np.zeros((NCORES, P, nch, P), dtype=ml_dtypes.bfloat16)
    dl_all = pl.dloc_stream.reshape(NCORES, nch, P)        # [c, g, e]
    wg_all = pl.wgt_stream.reshape(NCORES, nch, P)
    for c in range(NCORES):
        g_i, e_i = np.nonzero(dl_all[c] >= 0)
        eqs[c, e_i, g_i, dl_all[c, g_i, e_i].astype(np.int64)] = \
            wg_all[c, g_i, e_i].astype(ml_dtypes.bfloat16)

    mx = max(c for _, _, c in pl.calls) * P
    gi = np.zeros((NCORES, len(pl.calls), P, mx // 16), dtype=np.int16)
    for ci, (b, s0, c) in enumerate(pl.calls):
        seg = pl.idx_stream[:, s0 * P:(s0 + c) * P]
        for cc in range(NCORES):
            w = wrap_idx(seg[cc])
            gi[cc, ci, :, :w.shape[1]] = w

    in_maps = []
    for c in range(NCORES):
        in_maps.append({
            "tab1": tab1,
            "gidx": np.ascontiguousarray(gi[c]),
            "eqs": np.ascontiguousarray(eqs[c]),
            "W12": W12,
        })

    # host-side correction for nonzero b1/b2 (zero in this problem):
    # out += s[:,None]*(b1@W2)[None,:] + b2, s[n] = sum_{e->n} dinv_s*dinv_d
    corr = None
    if (b1 is not None and np.any(b1)) or (b2 is not None and np.any(b2)):
        s = np.zeros(n, np.float64)
        np.add.at(s, dst, wgt_e.astype(np.float64))
        corr = np.zeros((n, D), np.float32)
        if b1 is not None and np.any(b1):
            corr += s[:, None].astype(np.float32) * (np.asarray(b1, np.float64)
                                                     @ np.asarray(W2, np.float64)
                                                     ).astype(np.float32)[None, :]
        if b2 is not None and np.any(b2):
            corr += np.asarray(b2, np.float32)[None, :]
    return pl, in_maps, gpos, corr


def build_program(pl, reps=1):
    import os as _os
    DEBUG_L1 = _os.environ.get("K2_DEBUG_L1") == "1"
    DEBUG_L2 = _os.environ.get("K2_DEBUG_L2") == "1"
    NOMM = _os.environ.get("K2_NOMM") == "1"        # timing ablation only
    CONSTEQ = _os.environ.get("K2_CONSTEQ") == "1"  # timing ablation only
    bf16, f32, i16 = mybir.dt.bfloat16, mybir.dt.float32, mybir.dt.int16
    nch = pl.nchunk
    mxcall = max(c for _, _, c in pl.calls)
    er = CFG["elem_rows"]
    esize = er * D
    sp = CFG["single_packet"]

    nc = bacc.Bacc("TRN2", target_bir_lowering=False, debug=False,
                   num_devices=NCORES, num_swdge_queues=4)
    tab1_t = nc.dram_tensor("tab1", [NPAD + 2, D], bf16, kind="ExternalInput")
    gidx = nc.dram_tensor("gidx", [len(pl.calls), P, (mxcall * P) // 16], i16,
                          kind="ExternalInput").ap()
    eqs = nc.dram_tensor("eqs", [P, nch, P], bf16, kind="ExternalInput").ap()
    W12 = nc.dram_tensor("W12", [D, D], f32, kind="ExternalInput").ap()
    out = nc.dram_tensor("out", [SPAD, D], f32, kind="ExternalOutput").ap()
    dbg_acc = None
    if DEBUG_L1:
        dbg_acc = nc.dram_tensor("dbg_acc", [P, NWIN * P], f32,
                                 kind="ExternalOutput").ap()
    dbg_tab2 = None
    if DEBUG_L2:
        dbg_tab2 = nc.dram_tensor("dbg_tab2", [NPAD, D], bf16,
                                  kind="ExternalOutput").ap()

    ag_in = nc.dram_tensor("ag_in", [SPAD, D], bf16)
    tab2_t = nc.dram_tensor("tab2", [NPAD, D], bf16,
                            addr_space=CFG["tab2_space"])

    def tab_view(tensor, b, nrows_cap):
        """Bucket-b gather view with er-row elements (over-read trick)."""
        nrows = min(BUCKET, nrows_cap - b * BUCKET - (er - 1))
        if er == 1:
            return tensor.ap()[b * BUCKET: b * BUCKET + nrows, :]
        return AP(tensor, b * BUCKET * D, [[D, nrows], [1, esize]])

    with tile.TileContext(nc) as tc, ExitStack() as ctx:
        const = ctx.enter_context(tc.tile_pool(name="const", bufs=1))
        accp = ctx.enter_context(tc.tile_pool(name="accp", bufs=1))
        sbM = ctx.enter_context(tc.tile_pool(name="sbM", bufs=CFG["msg_bufs"]))
        sbE = ctx.enter_context(tc.tile_pool(name="sbE", bufs=4))
        sbO = ctx.enter_context(tc.tile_pool(name="sbO", bufs=4))
        ipP = ctx.enter_context(tc.tile_pool(name="ipP", bufs=3))
        psR = ctx.enter_context(tc.tile_pool(name="psR", bufs=4, space="PSUM"))
        psM = ctx.enter_context(tc.tile_pool(name="psM", bufs=2, space="PSUM"))

        nc.gpsimd.load_library(library_config.mlp)

        ident = const.tile([P, P], f32)
        make_identity(nc, ident[:])
        w12t = const.tile([P, P], f32, tag="w12t")
        nc.sync.dma_start(w12t[:], W12[:])
        zerot = const.tile([P, P], f32, tag="zerot")
        nc.vector.memset(zerot[:], 0.0)

        acc = accp.tile([P, NWIN * P], f32)

        for _rep in range(reps):
            nc.vector.memset(acc[:], 0.0)

            def tt_copy(dst_ap, src_ap):
                # tensor_tensor never enters DVE 2-port perf mode (which
                # would lock SWDGE out of its descriptor rings); plain
                # tensor_copy/tensor_scalar would stall concurrent gathers
                nc.vector.tensor_tensor(dst_ap, src_ap, zerot[:],
                                        mybir.AluOpType.add)

            def edge_pass(tab_tensor, nrows_cap, layer):
                qrr = [0]
                chunk_pos = 0
                open_run = {}
                drained = set()
                for ci, (b, s0, ncall) in enumerate(pl.calls):
                    it = ipP.tile([P, (mxcall * P) // 16], i16, tag="idx")
                    nc.sync.dma_start(it[:], gidx[ci])
                    msg = sbM.tile([P, mxcall, esize], bf16, tag="msg")
                    nc.gpsimd.dma_gather(
                        msg[:, :ncall, :], tab_view(tab_tensor, b, nrows_cap),
                        it[:, :(ncall * P) // 16],
                        ncall * P, ncall * P, esize,
                        elem_step=(D if er > 1 else None),
                        single_packet=sp, queue_num=qrr[0] % 4)
                    qrr[0] += 1
                    et = sbE.tile([P, mxcall, P], bf16, tag="eq")
                    if not CONSTEQ:
                        # ACT's HWDGE ring: parallel to sync's (idx/cast/out)
                        nc.scalar.dma_start(et[:, :ncall, :],
                                            eqs[:, s0:s0 + ncall, :])
                    if NOMM:
                        chunk_pos += ncall
                        continue
                    for j in range(ncall):
                        g = chunk_pos + j
                        bb, ww = pl.sched[g]
                        kk = int(pl.K[bb, ww])
                        off = g - pl.run_start[(bb, ww)]
                        if off == 0:
                            open_run[ww] = psR.tile([P, P], f32, tag="run",
                                                    name="runps")
                        ps = open_run[ww]
                        nc.tensor.matmul(ps[:], lhsT=msg[:, j, :D],
                                         rhs=et[:, j, :],
                                         start=(off == 0), stop=(off == kk - 1))
                        if off == kk - 1:
                            aw = acc[:, ww * P:(ww + 1) * P]
                            if ww in drained:
                                nc.vector.tensor_add(aw, aw, ps[:])
                            else:
                                tt_copy(aw, ps[:])
                                drained.add(ww)
                            del open_run[ww]
                            if pl.last_bucket[ww] == bb:
                                if layer == 1:
                                    # table rows must be node-major: transpose
                                    # the [feat, dst] acc window via PE
                                    tps = psM.tile([P, P], f32, tag="tps")
                                    nc.tensor.transpose(out=tps[:], in_=aw,
                                                        identity=ident[:])
                                    xws = sbO.tile([P, P], bf16, tag="xws")
                                    tt_copy(xws[:], tps[:])
                                    nc.sync.dma_start(
                                        ag_in.ap()[ww * P:(ww + 1) * P, :],
                                        xws[:])
                                else:
                                    ops = psM.tile([P, D], f32, tag="ops")
                                    nc.tensor.matmul(ops[:], lhsT=aw,
                                                     rhs=w12t[:],
                                                     start=True, stop=True)
                                    ot = sbO.tile([P, D], f32, tag="ot")
                                    tt_copy(ot[:], ops[:])
                                    nc.sync.dma_start(
                                        out[ww * P:(ww + 1) * P, :], ot[:])
                    chunk_pos += ncall

            edge_pass(tab1_t, NPAD + 2, 1)

            if DEBUG_L1:
                nc.sync.dma_start(dbg_acc[:], acc[:])
                continue

            nc.gpsimd.collective_compute(
                "AllGather", mybir.AluOpType.bypass,
                ins=[ag_in.ap()[:]], outs=[tab2_t.ap()[:]],
                replica_groups=[list(range(NCORES))])

            if DEBUG_L2:
                for bb in range(NB):
                    nc.sync.dma_start(
                        dbg_tab2[bb * BUCKET:(bb + 1) * BUCKET, :],
                        tab2_t.ap()[bb * BUCKET:(bb + 1) * BUCKET, :])
                continue

            # layer-2 drains overwrite acc (copy-first per window); Tile WAR
            # tracking orders them after the layer-1 casts
            edge_pass(tab2_t, NPAD, 2)

    nc.compile()
    return nc


_PROG_CACHE = {}


def run(x, edge_index, t_index, W1, W2, freq, phase=None, b1=None, b2=None,
        nc_prog=None):
    pl, in_maps, gpos, corr = preprocess(x, edge_index, t_index, W1, W2, freq,
                                         phase, b1, b2)
    if nc_prog is not None:
        nc = nc_prog
    else:
        key = (pl.nchunk, len(pl.calls), CFG["elem_rows"], CFG["gcall"])
        if key not in _PROG_CACHE:
            _PROG_CACHE[key] = build_program(pl)
        nc = _PROG_CACHE[key]
    res = run_bass_kernel_spmd(nc, in_maps, list(range(NCORES)))
    full = np.concatenate([res.results[c]["out"] for c in range(NCORES)], axis=0)
    out = full[gpos]
    if corr is not None:
        out = out + corr
    return np.ascontiguousarray(out.astype(np.float32))


def kernel(x, edge_index, t_index, W1, b1, W2, b2, freq, phase):
    x = np.asarray(x, dtype=np.float32)
    t_index = np.asarray(t_index, dtype=np.float32)
    return run(x, np.asarray(edge_index), t_index,
               np.asarray(W1, np.float32), np.asarray(W2, np.float32),
               np.asarray(freq, np.float32),
               phase=np.asarray(phase, np.float32),
               b1=np.asarray(b1, np.float32), b2=np.asarray(b2, np.float32))


# revision 4
# speedup vs baseline: 1.0438x; 1.0002x over previous
"""Trainium2 Bass kernel v2 for nn_DiGCN (2-layer GCN + TimeEncode), 8 cores.

Key restructuring vs v1:
- b1=b2=0 => h2 = A2(A2 h W1)W2 = (A2^2 h)(W1@W2): the dense transforms
  commute out. Device does two gather/segment-sum passes + one final 128x128
  matmul per window. No phase A, no per-layer W matmuls, no dinv scaling
  (dinv[src]*dinv[dst] folded into the per-edge one-hot weights on host).
- h = x + cos(t*freq + phase) precomputed on host as the layer-1 gather
  table (bf16, node-major, padded block layout) - an ExternalInput.
- Block node ownership: core c owns padded rows [c*12544, (c+1)*12544);
  ONE AllGather of the layer-1 result (bf16) builds the layer-2 table.
- Gather config tunable (elem_rows over-read trick for 512B descriptors,
  src-sorted streams) per microbenchmark results.
"""
import math
import numpy as np

import sys
if "/opt/trn_rl_repo" not in sys.path:
    sys.path.insert(0, "/opt/trn_rl_repo")

from contextlib import ExitStack

import concourse.bass as bass
import concourse.tile as tile
from concourse.bass_types import AP
from concourse import bacc, mybir
from concourse.bass_utils import run_bass_kernel_spmd
from concourse import library_config
from concourse.masks import make_identity

P = 128
NCORES = 8
D = 128
N_NODES = 100000
REAL_PER_CORE = 12500
SPAD = 12544                     # per-core padded block (98 windows)
NPAD = SPAD * NCORES             # 100352
BUCKET = 25088                   # gather bucket rows (int16-addressable)
NB = NPAD // BUCKET              # 4
NWIN = SPAD // P                 # 98

# gather config (tuned via gather_bench: unsorted 256B rows from private
# tables hit ~211 GB/s/core; run-sorted and over-read variants were slower)
CFG = {
    "elem_rows": 1,      # table rows per descriptor
    "gcall": 8192,       # indices per gather call
    "sort": False,       # (b,w)-run src sort benched SLOWER than unsorted
    "single_packet": False,
    "msg_bufs": 4,
    "tab2_space": "Local",
}


class Plan:
    pass


def wrap_idx(idx):
    """[n] -> [128, n/16] int16: idx wrapped into 16 partitions, tiled 8x."""
    n = len(idx)
    a = idx.reshape(n // 16, 16).T
    return np.ascontiguousarray(np.tile(a, (8, 1))).astype(np.int16)


def build_plan(src_gpos, dst_gpos, wgt_e, gcall):
    pl = Plan()
    e_sb = (src_gpos // BUCKET).astype(np.int32)
    e_idx = (src_gpos % BUCKET).astype(np.int32)
    e_core = (dst_gpos // SPAD).astype(np.int32)
    dpos = dst_gpos % SPAD
    e_w = (dpos // P).astype(np.int32)
    e_dloc = (dpos % P).astype(np.int32)

    counts = np.zeros((NCORES, NB, NWIN), dtype=np.int64)
    np.add.at(counts, (e_core, e_sb, e_w), 1)
    K = np.ceil(counts / P).astype(np.int64).max(axis=0)    # [NB, NWIN]
    assert (K.sum(axis=0) > 0).all(), "window with no chunks"
    pl.K = K
    pl.nchunk = int(K.sum())
    pl.border = list(range(NB))

    # schedule: (bucket, window) repeated K times, bucket-major
    sched = []
    for b in pl.border:
        for w in range(NWIN):
            sched.extend([(b, w)] * int(K[b, w]))
    pl.sched = sched

    # run start offsets (chunk index of first chunk of each (b,w) run)
    starts = {}
    acc = 0
    for b in pl.border:
        for w in range(NWIN):
            starts[(b, w)] = acc
            acc += int(K[b, w])
    pl.run_start = starts

    # last bucket (in border order) with chunks for each window
    lastb = {}
    for b in pl.border:
        for w in range(NWIN):
            if K[b, w] > 0:
                lastb[w] = b
    pl.last_bucket = lastb

    # gather calls: per bucket, chunks split into calls of <= gcall/P chunks
    pl.gcall = gcall
    calls = []
    pos = 0
    for b in pl.border:
        cb = int(K[b].sum())
        s = 0
        while s < cb:
            c = min(gcall // P, cb - s)
            calls.append((b, pos + s, c))
            s += c
        pos += cb
    pl.calls = calls

    # per-core streams in schedule order
    if CFG["sort"]:
        order = np.lexsort((e_idx, e_w, e_sb, e_core))
    else:
        order = np.lexsort((e_w, e_sb, e_core))
    osrc, ow, odloc, owgt, ocore, osb = (
        e_idx[order], e_w[order], e_dloc[order], wgt_e[order],
        e_core[order], e_sb[order])

    nch = pl.nchunk
    # pad slots get RANDOM in-bucket indices (dloc=-1 zeroes their weight):
    # constant-0 pads would make ~12% of all descriptors hit one 256B row —
    # an HBM bank hotspot shared by all 8 cores
    rng = np.random.default_rng(12345)
    idx_s = rng.integers(0, BUCKET - 1, (NCORES, nch * P)).astype(np.int16)
    dloc_s = np.full((NCORES, nch * P), -1.0, dtype=np.float32)
    wgt_s = np.zeros((NCORES, nch * P), dtype=np.float32)

    chunk_off = np.zeros((NB, NWIN), dtype=np.int64)
    for (b, w), st in starts.items():
        chunk_off[b, w] = st

    ptr = np.searchsorted(ocore, np.arange(NCORES + 1))
    for c in range(NCORES):
        lo, hi = ptr[c], ptr[c + 1]
        csb, cw = osb[lo:hi], ow[lo:hi]
        keys = csb.astype(np.int64) * NWIN + cw
        uq, inv, cnts = np.unique(keys, return_inverse=True, return_counts=True)
        grp_start = np.concatenate([[0], np.cumsum(cnts)])[:-1]
        local = np.arange(hi - lo) - grp_start[inv]
        gpos = (chunk_off[csb, cw] * P + local).astype(np.int64)
        idx_s[c, gpos] = osrc[lo:hi].astype(np.int16)
        dloc_s[c, gpos] = odloc[lo:hi].astype(np.float32)
        wgt_s[c, gpos] = owgt[lo:hi]
        # pad slots duplicate REAL indices of the same (b,w) run: those rows
        # are already fetched by this chunk/run, so the extra descriptors hit
        # the HBM row buffer instead of being independent random reads
        ar = np.arange(P)
        for ui, key in enumerate(uq):
            b, w = int(key) // NWIN, int(key) % NWIN
            cnt = int(cnts[ui])
            base = int(chunk_off[b, w])
            kbw = int(K[b, w])
            for o in range(kbw):
                r = min(max(cnt - o * P, 0), P)
                if r == P:
                    continue
                g0 = (base + o) * P
                if r > 0:
                    idx_s[c, g0 + r:g0 + P] = idx_s[c, g0 + (ar[r:] % r)]
                else:
                    rr = min(cnt, P)
                    if rr > 0:
                        idx_s[c, g0:g0 + P] = idx_s[c, base * P + (ar % rr)]

    pl.idx_stream = idx_s
    pl.dloc_stream = dloc_s
    pl.wgt_stream = wgt_s
    return pl


def preprocess(x, edge_index, t_index, W1, W2, freq,
               phase=None, b1=None, b2=None, gcall=None):
    n = x.shape[0]
    assert n == N_NODES
    gcall = gcall or CFG["gcall"]
    x = np.asarray(x, np.float32)
    t_index = np.asarray(t_index, np.float32)
    freq = np.asarray(freq, np.float32)
    phase_a = np.zeros(D, np.float32) if phase is None else np.asarray(phase, np.float32)

    src = np.asarray(edge_index[0], dtype=np.int64)
    dst = np.asarray(edge_index[1], dtype=np.int64)
    loop = np.arange(n, dtype=np.int64)
    src = np.concatenate([src, loop])
    dst = np.concatenate([dst, loop])
    deg = np.bincount(dst, minlength=n).astype(np.float64)
    dinv = np.where(deg > 0, 1.0 / np.sqrt(np.maximum(deg, 1e-12)), 0.0)
    dinv = dinv.astype(np.float32)

    # node -> padded global position
    nid = np.arange(n, dtype=np.int64)
    core = nid // REAL_PER_CORE
    gpos = core * SPAD + (nid - core * REAL_PER_CORE)

    wgt_e = (dinv[src] * dinv[dst]).astype(np.float32)
    pl = build_plan(gpos[src], gpos[dst], wgt_e, gcall)

    # layer-1 table: h = x + cos(t freq + phase), padded block layout (+pad
    # rows for the multi-row over-read)
    h = x + np.cos(t_index[:, None] * freq[None, :] + phase_a[None, :])
    import ml_dtypes
    tab1 = np.zeros((NPAD + 2, D), dtype=ml_dtypes.bfloat16)
    tab1[gpos] = h.astype(ml_dtypes.bfloat16)

    W12 = (np.asarray(W1, np.float64) @ np.asarray(W2, np.float64)).astype(np.float32)

    nch = pl.nchunk

    # host-built one-hot eq stream, partition-major [P(edge), nch, P(dst)]
    # bf16: eqs[e, g, dloc[e,g]] = wgt[e,g]. Streamed per call via HWDGE
    # (contiguous 16KB/partition reads) — per-chunk DVE tensor_scalar builds
    # would lock SWDGE out of its descriptor rings (2-port perf mode).
    eqs = np.zeros((NCORES, P, nch, P), dtype=ml_dtypes.bfloat16)
    dl_all = pl.dloc_stream.reshape(NCORES, nch, P)        # [c, g, e]
    wg_all = pl.wgt_stream.reshape(NCORES, nch, P)
    for c in range(NCORES):
        g_i, e_i = np.nonzero(dl_all[c] >= 0)
        eqs[c, e_i, g_i, dl_all[c, g_i, e_i].astype(np.int64)] = \
            wg_all[c, g_i, e_i].astype(ml_dtypes.bfloat16)

    mx = max(c for _, _, c in pl.calls) * P
    gi = np.zeros((NCORES, len(pl.calls), P, mx // 16), dtype=np.int16)
    for ci, (b, s0, c) in enumerate(pl.calls):
        seg = pl.idx_stream[:, s0 * P:(s0 + c) * P]
        for cc in range(NCORES):
            w = wrap_idx(seg[cc])
            gi[cc, ci, :, :w.shape[1]] = w

    in_maps = []
    for c in range(NCORES):
        in_maps.append({
            "tab1": tab1,
            "gidx": np.ascontiguousarray(gi[c]),
            "eqs": np.ascontiguousarray(eqs[c]),
            "W12": W12,
        })

    # host-side correction for nonzero b1/b2 (zero in this problem):
    # out += s[:,None]*(b1@W2)[None,:] + b2, s[n] = sum_{e->n} dinv_s*dinv_d
    corr = None
    if (b1 is not None and np.any(b1)) or (b2 is not None and np.any(b2)):
        s = np.zeros(n, np.float64)
        np.add.at(s, dst, wgt_e.astype(np.float64))
        corr = np.zeros((n, D), np.float32)
        if b1 is not None and np.any(b1):
            corr += s[:, None].astype(np.float32) * (np.asarray(b1, np.float64)
                                                     @ np.asarray(W2, np.float64)
                                                     ).astype(np.float32)[None, :]
        if b2 is not None and np.any(b2):
            corr += np.asarray(b2, np.float32)[None, :]
    return pl, in_maps, gpos, corr


def build_program(pl, reps=1):
    import os as _os
    DEBUG_L1 = _os.environ.get("K2_DEBUG_L1") == "1"
    DEBUG_L2 = _os.environ.get("K2_DEBUG_L2") == "1"
    NOMM = _os.environ.get("K2_NOMM") == "1"        # timing ablation only
    CONSTEQ = _os.environ.get("K2_CONSTEQ") == "1"  # timing ablation only
    bf16, f32, i16 = mybir.dt.bfloat16, mybir.dt.float32, mybir.dt.int16
    nch = pl.nchunk
    mxcall = max(c for _, _, c in pl.calls)
    er = CFG["elem_rows"]
    esize = er * D
    sp = CFG["single_packet"]

    nc = bacc.Bacc("TRN2", target_bir_lowering=False, debug=False,
                   num_devices=NCORES, num_swdge_queues=4)
    tab1_t = nc.dram_tensor("tab1", [NPAD + 2, D], bf16, kind="ExternalInput")
    gidx = nc.dram_tensor("gidx", [len(pl.calls), P, (mxcall * P) // 16], i16,
                          kind="ExternalInput").ap()
    eqs = nc.dram_tensor("eqs", [P, nch, P], bf16, kind="ExternalInput").ap()
    W12 = nc.dram_tensor("W12", [D, D], f32, kind="ExternalInput").ap()
    out = nc.dram_tensor("out", [SPAD, D], f32, kind="ExternalOutput").ap()
    dbg_acc = None
    if DEBUG_L1:
        dbg_acc = nc.dram_tensor("dbg_acc", [P, NWIN * P], f32,
                                 kind="ExternalOutput").ap()
    dbg_tab2 = None
    if DEBUG_L2:
        dbg_tab2 = nc.dram_tensor("dbg_tab2", [NPAD, D], bf16,
                                  kind="ExternalOutput").ap()

    ag_in = nc.dram_tensor("ag_in", [SPAD, D], bf16)
    tab2_t = nc.dram_tensor("tab2", [NPAD, D], bf16,
                            addr_space=CFG["tab2_space"])

    def tab_view(tensor, b, nrows_cap):
        """Bucket-b gather view with er-row elements (over-read trick)."""
        nrows = min(BUCKET, nrows_cap - b * BUCKET - (er - 1))
        if er == 1:
            return tensor.ap()[b * BUCKET: b * BUCKET + nrows, :]
        return AP(tensor, b * BUCKET * D, [[D, nrows], [1, esize]])

    with tile.TileContext(nc) as tc, ExitStack() as ctx:
        const = ctx.enter_context(tc.tile_pool(name="const", bufs=1))
        accp = ctx.enter_context(tc.tile_pool(name="accp", bufs=1))
        sbM = ctx.enter_context(tc.tile_pool(name="sbM", bufs=CFG["msg_bufs"]))
        sbE = ctx.enter_context(tc.tile_pool(name="sbE", bufs=4))
        sbO = ctx.enter_context(tc.tile_pool(name="sbO", bufs=4))
        ipP = ctx.enter_context(tc.tile_pool(name="ipP", bufs=3))
        psR = ctx.enter_context(tc.tile_pool(name="psR", bufs=4, space="PSUM"))
        psM = ctx.enter_context(tc.tile_pool(name="psM", bufs=2, space="PSUM"))

        nc.gpsimd.load_library(library_config.mlp)

        ident = const.tile([P, P], f32)
        make_identity(nc, ident[:])
        w12t = const.tile([P, P], f32, tag="w12t")
        nc.sync.dma_start(w12t[:], W12[:])
        zerot = const.tile([P, P], f32, tag="zerot")
        nc.vector.memset(zerot[:], 0.0)

        acc = accp.tile([P, NWIN * P], f32)

        for _rep in range(reps):
            nc.vector.memset(acc[:], 0.0)

            def tt_copy(dst_ap, src_ap):
                # tensor_tensor never enters DVE 2-port perf mode (which
                # would lock SWDGE out of its descriptor rings); plain
                # tensor_copy/tensor_scalar would stall concurrent gathers
                nc.vector.tensor_tensor(dst_ap, src_ap, zerot[:],
                                        mybir.AluOpType.add)

            def edge_pass(tab_tensor, nrows_cap, layer):
                qrr = [0]
                chunk_pos = 0
                open_run = {}
                drained = set()
                for ci, (b, s0, ncall) in enumerate(pl.calls):
                    it = ipP.tile([P, (mxcall * P) // 16], i16, tag="idx")
                    nc.sync.dma_start(it[:], gidx[ci])
                    msg = sbM.tile([P, mxcall, esize], bf16, tag="msg")
                    nc.gpsimd.dma_gather(
                        msg[:, :ncall, :], tab_view(tab_tensor, b, nrows_cap),
                        it[:, :(ncall * P) // 16],
                        ncall * P, ncall * P, esize,
                        elem_step=(D if er > 1 else None),
                        single_packet=sp, queue_num=qrr[0] % 4)
                    qrr[0] += 1
                    et = sbE.tile([P, mxcall, P], bf16, tag="eq")
                    if not CONSTEQ:
                        # ACT's HWDGE ring: parallel to sync's (idx/cast/out)
                        nc.scalar.dma_start(et[:, :ncall, :],
                                            eqs[:, s0:s0 + ncall, :])
                    if NOMM:
                        chunk_pos += ncall
                        continue
                    for j in range(ncall):
                        g = chunk_pos + j
                        bb, ww = pl.sched[g]
                        kk = int(pl.K[bb, ww])
                        off = g - pl.run_start[(bb, ww)]
                        if off == 0:
                            open_run[ww] = psR.tile([P, P], f32, tag="run",
                                                    name="runps")
                        ps = open_run[ww]
                        nc.tensor.matmul(ps[:], lhsT=msg[:, j, :D],
                                         rhs=et[:, j, :],
                                         start=(off == 0), stop=(off == kk - 1))
                        if off == kk - 1:
                            aw = acc[:, ww * P:(ww + 1) * P]
                            if ww in drained:
                                nc.vector.tensor_add(aw, aw, ps[:])
                            else:
                                tt_copy(aw, ps[:])
                                drained.add(ww)
                            del open_run[ww]
                            if pl.last_bucket[ww] == bb:
                                if layer == 1:
                                    # table rows must be node-major: transpose
                                    # the [feat, dst] acc window via PE
                                    tps = psM.tile([P, P], f32, tag="tps")
                                    nc.tensor.transpose(out=tps[:], in_=aw,
                                                        identity=ident[:])
                                    xws = sbO.tile([P, P], bf16, tag="xws")
                                    tt_copy(xws[:], tps[:])
                                    nc.sync.dma_start(
                                        ag_in.ap()[ww * P:(ww + 1) * P, :],
                                        xws[:])
                                else:
                                    ops = psM.tile([P, D], f32, tag="ops")
                                    nc.tensor.matmul(ops[:], lhsT=aw,
                                                     rhs=w12t[:],
                                                     start=True, stop=True)
                                    ot = sbO.tile([P, D], f32, tag="ot")
                                    tt_copy(ot[:], ops[:])
                                    nc.sync.dma_start(
                                        out[ww * P:(ww + 1) * P, :], ot[:])
                    chunk_pos += ncall

            edge_pass(tab1_t, NPAD + 2, 1)

            if DEBUG_L1:
                nc.sync.dma_start(dbg_acc[:], acc[:])
                continue

            nc.gpsimd.collective_compute(
                "AllGather", mybir.AluOpType.bypass,
                ins=[ag_in.ap()[:]], outs=[tab2_t.ap()[:]],
                replica_groups=[list(range(NCORES))])

            if DEBUG_L2:
                for bb in range(NB):
                    nc.sync.dma_start(
                        dbg_tab2[bb * BUCKET:(bb + 1) * BUCKET, :],
                        tab2_t.ap()[bb * BUCKET:(bb + 1) * BUCKET, :])
                continue

            # layer-2 drains overwrite acc (copy-first per window); Tile WAR
            # tracking orders them after the layer-1 casts
            edge_pass(tab2_t, NPAD, 2)

    nc.compile()
    return nc


_PROG_CACHE = {}


def run(x, edge_index, t_index, W1, W2, freq, phase=None, b1=None, b2=None,
        nc_prog=None):
    pl, in_maps, gpos, corr = preprocess(x, edge_index, t_index, W1, W2, freq,
                                         phase, b1, b2)
    if nc_prog is not None:
        nc = nc_prog
    else:
        key = (pl.nchunk, len(pl.calls), CFG["elem_rows"], CFG["gcall"])
        if key not in _PROG_CACHE:
            _PROG_CACHE[key] = build_program(pl)
        nc = _PROG_CACHE[key]
    res = run_bass_kernel_spmd(nc, in_maps, list(range(NCORES)))
    full = np.concatenate([res.results[c]["out"] for c in range(NCORES)], axis=0)
    out = full[gpos]
    if corr is not None:
        out = out + corr
    return np.ascontiguousarray(out.astype(np.float32))


def kernel(x, edge_index, t_index, W1, b1, W2, b2, freq, phase):
    x = np.asarray(x, dtype=np.float32)
    t_index = np.asarray(t_index, dtype=np.float32)
    return run(x, np.asarray(edge_index), t_index,
               np.asarray(W1, np.float32), np.asarray(W2, np.float32),
               np.asarray(freq, np.float32),
               phase=np.asarray(phase, np.float32),
               b1=np.asarray(b1, np.float32), b2=np.asarray(b2, np.float32))


# revision 5
# speedup vs baseline: 1.4669x; 1.4053x over previous
"""Trainium2 Bass kernel v2 for nn_DiGCN (2-layer GCN + TimeEncode), 8 cores.

Key restructuring vs v1:
- b1=b2=0 => h2 = A2(A2 h W1)W2 = (A2^2 h)(W1@W2): the dense transforms
  commute out. Device does two gather/segment-sum passes + one final 128x128
  matmul per window. No phase A, no per-layer W matmuls, no dinv scaling
  (dinv[src]*dinv[dst] folded into the per-edge one-hot weights on host).
- h = x + cos(t*freq + phase) precomputed on host as the layer-1 gather
  table (bf16, node-major, padded block layout) - an ExternalInput.
- Block node ownership: core c owns padded rows [c*12544, (c+1)*12544);
  ONE AllGather of the layer-1 result (bf16) builds the layer-2 table.
- Gather config tunable (elem_rows over-read trick for 512B descriptors,
  src-sorted streams) per microbenchmark results.
"""
import math
import numpy as np

import sys
if "/opt/trn_rl_repo" not in sys.path:
    sys.path.insert(0, "/opt/trn_rl_repo")

from contextlib import ExitStack

import concourse.bass as bass
import concourse.tile as tile
from concourse.bass_types import AP
from concourse import bacc, mybir
from concourse.bass_utils import run_bass_kernel_spmd
from concourse import library_config
from concourse.masks import make_identity

P = 128
NCORES = 8
D = 128
N_NODES = 100000
REAL_PER_CORE = 12500
SPAD = 12544                     # per-core padded block (98 windows)
NPAD = SPAD * NCORES             # 100352
BUCKET = 25088                   # gather bucket rows (int16-addressable)
NB = NPAD // BUCKET              # 4
NWIN = SPAD // P                 # 98

# gather config (tuned via gather_bench: unsorted 256B rows from private
# tables hit ~211 GB/s/core; run-sorted and over-read variants were slower)
CFG = {
    "elem_rows": 1,      # table rows per descriptor
    "gcall": 8192,       # indices per gather call
    "sort": False,       # (b,w)-run src sort benched SLOWER than unsorted
    "single_packet": False,
    "msg_bufs": 5,
    "tab2_space": "Local",
}


class Plan:
    pass


def wrap_idx(idx):
    """[n] -> [128, n/16] int16: idx wrapped into 16 partitions, tiled 8x."""
    n = len(idx)
    a = idx.reshape(n // 16, 16).T
    return np.ascontiguousarray(np.tile(a, (8, 1))).astype(np.int16)


def build_plan(src_gpos, dst_gpos, wgt_e, gcall):
    pl = Plan()
    e_sb = (src_gpos // BUCKET).astype(np.int32)
    e_idx = (src_gpos % BUCKET).astype(np.int32)
    e_core = (dst_gpos // SPAD).astype(np.int32)
    dpos = dst_gpos % SPAD
    e_w = (dpos // P).astype(np.int32)
    e_dloc = (dpos % P).astype(np.int32)

    counts = np.zeros((NCORES, NB, NWIN), dtype=np.int64)
    np.add.at(counts, (e_core, e_sb, e_w), 1)
    K = np.ceil(counts / P).astype(np.int64).max(axis=0)    # [NB, NWIN]
    assert (K.sum(axis=0) > 0).all(), "window with no chunks"
    pl.K = K
    pl.nchunk = int(K.sum())
    pl.border = list(range(NB))

    # schedule: (bucket, window) repeated K times, bucket-major
    sched = []
    for b in pl.border:
        for w in range(NWIN):
            sched.extend([(b, w)] * int(K[b, w]))
    pl.sched = sched

    # run start offsets (chunk index of first chunk of each (b,w) run)
    starts = {}
    acc = 0
    for b in pl.border:
        for w in range(NWIN):
            starts[(b, w)] = acc
            acc += int(K[b, w])
    pl.run_start = starts

    # last bucket (in border order) with chunks for each window
    lastb = {}
    for b in pl.border:
        for w in range(NWIN):
            if K[b, w] > 0:
                lastb[w] = b
    pl.last_bucket = lastb

    # gather calls: per bucket, chunks split into calls of <= gcall/P chunks
    pl.gcall = gcall
    calls = []
    pos = 0
    for b in pl.border:
        cb = int(K[b].sum())
        s = 0
        while s < cb:
            c = min(gcall // P, cb - s)
            calls.append((b, pos + s, c))
            s += c
        pos += cb
    pl.calls = calls

    # per-core streams in schedule order
    if CFG["sort"]:
        order = np.lexsort((e_idx, e_w, e_sb, e_core))
    else:
        order = np.lexsort((e_w, e_sb, e_core))
    osrc, ow, odloc, owgt, ocore, osb = (
        e_idx[order], e_w[order], e_dloc[order], wgt_e[order],
        e_core[order], e_sb[order])

    nch = pl.nchunk
    # pad slots get RANDOM in-bucket indices (dloc=-1 zeroes their weight):
    # constant-0 pads would make ~12% of all descriptors hit one 256B row —
    # an HBM bank hotspot shared by all 8 cores
    rng = np.random.default_rng(12345)
    idx_s = rng.integers(0, BUCKET - 1, (NCORES, nch * P)).astype(np.int16)
    dloc_s = np.full((NCORES, nch * P), -1.0, dtype=np.float32)
    wgt_s = np.zeros((NCORES, nch * P), dtype=np.float32)

    chunk_off = np.zeros((NB, NWIN), dtype=np.int64)
    for (b, w), st in starts.items():
        chunk_off[b, w] = st

    ptr = np.searchsorted(ocore, np.arange(NCORES + 1))
    for c in range(NCORES):
        lo, hi = ptr[c], ptr[c + 1]
        csb, cw = osb[lo:hi], ow[lo:hi]
        keys = csb.astype(np.int64) * NWIN + cw
        uq, inv, cnts = np.unique(keys, return_inverse=True, return_counts=True)
        grp_start = np.concatenate([[0], np.cumsum(cnts)])[:-1]
        local = np.arange(hi - lo) - grp_start[inv]
        gpos = (chunk_off[csb, cw] * P + local).astype(np.int64)
        idx_s[c, gpos] = osrc[lo:hi].astype(np.int16)
        dloc_s[c, gpos] = odloc[lo:hi].astype(np.float32)
        wgt_s[c, gpos] = owgt[lo:hi]
        # pad slots duplicate REAL indices of the same (b,w) run: those rows
        # are already fetched by this chunk/run, so the extra descriptors hit
        # the HBM row buffer instead of being independent random reads
        ar = np.arange(P)
        for ui, key in enumerate(uq):
            b, w = int(key) // NWIN, int(key) % NWIN
            cnt = int(cnts[ui])
            base = int(chunk_off[b, w])
            kbw = int(K[b, w])
            for o in range(kbw):
                r = min(max(cnt - o * P, 0), P)
                if r == P:
                    continue
                g0 = (base + o) * P
                if r > 0:
                    idx_s[c, g0 + r:g0 + P] = idx_s[c, g0 + (ar[r:] % r)]
                else:
                    rr = min(cnt, P)
                    if rr > 0:
                        idx_s[c, g0:g0 + P] = idx_s[c, base * P + (ar % rr)]

    pl.idx_stream = idx_s
    pl.dloc_stream = dloc_s
    pl.wgt_stream = wgt_s
    return pl


def preprocess(x, edge_index, t_index, W1, W2, freq,
               phase=None, b1=None, b2=None, gcall=None):
    n = x.shape[0]
    assert n == N_NODES
    gcall = gcall or CFG["gcall"]
    x = np.asarray(x, np.float32)
    t_index = np.asarray(t_index, np.float32)
    freq = np.asarray(freq, np.float32)
    phase_a = np.zeros(D, np.float32) if phase is None else np.asarray(phase, np.float32)

    src = np.asarray(edge_index[0], dtype=np.int64)
    dst = np.asarray(edge_index[1], dtype=np.int64)
    loop = np.arange(n, dtype=np.int64)
    src = np.concatenate([src, loop])
    dst = np.concatenate([dst, loop])
    deg = np.bincount(dst, minlength=n).astype(np.float64)
    dinv = np.where(deg > 0, 1.0 / np.sqrt(np.maximum(deg, 1e-12)), 0.0)
    dinv = dinv.astype(np.float32)

    # node -> padded global position
    nid = np.arange(n, dtype=np.int64)
    core = nid // REAL_PER_CORE
    gpos = core * SPAD + (nid - core * REAL_PER_CORE)

    wgt_e = (dinv[src] * dinv[dst]).astype(np.float32)
    pl = build_plan(gpos[src], gpos[dst], wgt_e, gcall)

    # layer-1 table: h = x + cos(t freq + phase), padded block layout (+pad
    # rows for the multi-row over-read)
    h = x + np.cos(t_index[:, None] * freq[None, :] + phase_a[None, :])
    import ml_dtypes
    tab1 = np.zeros((NPAD + 2, D), dtype=ml_dtypes.bfloat16)
    tab1[gpos] = h.astype(ml_dtypes.bfloat16)

    W12 = (np.asarray(W1, np.float64) @ np.asarray(W2, np.float64)).astype(np.float32)

    nch = pl.nchunk

    # host-built one-hot eq stream, partition-major [P(edge), nch, P(dst)]
    # bf16: eqs[e, g, dloc[e,g]] = wgt[e,g]. Streamed per call via HWDGE
    # (contiguous 16KB/partition reads) — per-chunk DVE tensor_scalar builds
    # would lock SWDGE out of its descriptor rings (2-port perf mode).
    eqs = np.zeros((NCORES, P, nch, P), dtype=ml_dtypes.bfloat16)
    dl_all = pl.dloc_stream.reshape(NCORES, nch, P)        # [c, g, e]
    wg_all = pl.wgt_stream.reshape(NCORES, nch, P)
    for c in range(NCORES):
        g_i, e_i = np.nonzero(dl_all[c] >= 0)
        eqs[c, e_i, g_i, dl_all[c, g_i, e_i].astype(np.int64)] = \
            wg_all[c, g_i, e_i].astype(ml_dtypes.bfloat16)

    mx = max(c for _, _, c in pl.calls) * P
    gi = np.zeros((NCORES, len(pl.calls), P, mx // 16), dtype=np.int16)
    for ci, (b, s0, c) in enumerate(pl.calls):
        seg = pl.idx_stream[:, s0 * P:(s0 + c) * P]
        for cc in range(NCORES):
            w = wrap_idx(seg[cc])
            gi[cc, ci, :, :w.shape[1]] = w

    in_maps = []
    for c in range(NCORES):
        in_maps.append({
            "tab1": tab1,
            "gidx": np.ascontiguousarray(gi[c]),
            "eqs": np.ascontiguousarray(eqs[c]),
            "W12": W12,
        })

    # host-side correction for nonzero b1/b2 (zero in this problem):
    # out += s[:,None]*(b1@W2)[None,:] + b2, s[n] = sum_{e->n} dinv_s*dinv_d
    corr = None
    if (b1 is not None and np.any(b1)) or (b2 is not None and np.any(b2)):
        s = np.zeros(n, np.float64)
        np.add.at(s, dst, wgt_e.astype(np.float64))
        corr = np.zeros((n, D), np.float32)
        if b1 is not None and np.any(b1):
            corr += s[:, None].astype(np.float32) * (np.asarray(b1, np.float64)
                                                     @ np.asarray(W2, np.float64)
                                                     ).astype(np.float32)[None, :]
        if b2 is not None and np.any(b2):
            corr += np.asarray(b2, np.float32)[None, :]
    return pl, in_maps, gpos, corr


def build_program(pl, reps=1):
    import os as _os
    DEBUG_L1 = _os.environ.get("K2_DEBUG_L1") == "1"
    DEBUG_L2 = _os.environ.get("K2_DEBUG_L2") == "1"
    NOMM = _os.environ.get("K2_NOMM") == "1"        # timing ablation only
    CONSTEQ = _os.environ.get("K2_CONSTEQ") == "1"  # timing ablation only
    bf16, f32, i16 = mybir.dt.bfloat16, mybir.dt.float32, mybir.dt.int16
    nch = pl.nchunk
    mxcall = max(c for _, _, c in pl.calls)
    er = CFG["elem_rows"]
    esize = er * D
    sp = CFG["single_packet"]

    nc = bacc.Bacc("TRN2", target_bir_lowering=False, debug=False,
                   num_devices=NCORES, num_swdge_queues=4)
    tab1_t = nc.dram_tensor("tab1", [NPAD + 2, D], bf16, kind="ExternalInput")
    gidx = nc.dram_tensor("gidx", [len(pl.calls), P, (mxcall * P) // 16], i16,
                          kind="ExternalInput").ap()
    eqs = nc.dram_tensor("eqs", [P, nch, P], bf16, kind="ExternalInput").ap()
    W12 = nc.dram_tensor("W12", [D, D], f32, kind="ExternalInput").ap()
    out = nc.dram_tensor("out", [SPAD, D], f32, kind="ExternalOutput").ap()
    dbg_acc = None
    if DEBUG_L1:
        dbg_acc = nc.dram_tensor("dbg_acc", [P, NWIN * P], f32,
                                 kind="ExternalOutput").ap()
    dbg_tab2 = None
    if DEBUG_L2:
        dbg_tab2 = nc.dram_tensor("dbg_tab2", [NPAD, D], bf16,
                                  kind="ExternalOutput").ap()

    ag_in = nc.dram_tensor("ag_in", [SPAD, D], bf16)
    tab2_t = nc.dram_tensor("tab2", [NPAD, D], bf16,
                            addr_space=CFG["tab2_space"])

    def tab_view(tensor, b, nrows_cap):
        """Bucket-b gather view with er-row elements (over-read trick)."""
        nrows = min(BUCKET, nrows_cap - b * BUCKET - (er - 1))
        if er == 1:
            return tensor.ap()[b * BUCKET: b * BUCKET + nrows, :]
        return AP(tensor, b * BUCKET * D, [[D, nrows], [1, esize]])

    with tile.TileContext(nc) as tc, ExitStack() as ctx:
        const = ctx.enter_context(tc.tile_pool(name="const", bufs=1))
        accp = ctx.enter_context(tc.tile_pool(name="accp", bufs=1))
        sbM = ctx.enter_context(tc.tile_pool(name="sbM", bufs=CFG["msg_bufs"]))
        sbE = ctx.enter_context(tc.tile_pool(name="sbE", bufs=3))
        sbO = ctx.enter_context(tc.tile_pool(name="sbO", bufs=4))
        ipP = ctx.enter_context(tc.tile_pool(name="ipP", bufs=6))
        psR = ctx.enter_context(tc.tile_pool(name="psR", bufs=4, space="PSUM"))
        psM = ctx.enter_context(tc.tile_pool(name="psM", bufs=2, space="PSUM"))

        nc.gpsimd.load_library(library_config.mlp)

        ident = const.tile([P, P], f32)
        make_identity(nc, ident[:])
        w12t = const.tile([P, P], f32, tag="w12t")
        nc.sync.dma_start(w12t[:], W12[:])
        zerot = const.tile([P, P], f32, tag="zerot")
        nc.vector.memset(zerot[:], 0.0)

        acc = accp.tile([P, NWIN * P], f32)

        for _rep in range(reps):
            nc.vector.memset(acc[:], 0.0)

            def tt_copy(dst_ap, src_ap):
                # tensor_tensor never enters DVE 2-port perf mode (which
                # would lock SWDGE out of its descriptor rings); plain
                # tensor_copy/tensor_scalar would stall concurrent gathers
                nc.vector.tensor_tensor(dst_ap, src_ap, zerot[:],
                                        mybir.AluOpType.add)

            def edge_pass(tab_tensor, nrows_cap, layer):
                qrr = [0]
                chunk_pos = 0
                open_run = {}
                drained = set()
                for ci, (b, s0, ncall) in enumerate(pl.calls):
                    it = ipP.tile([P, (mxcall * P) // 16], i16, tag="idx")
                    nc.sync.dma_start(it[:], gidx[ci])
                    msg = sbM.tile([P, mxcall, esize], bf16, tag="msg")
                    nc.gpsimd.dma_gather(
                        msg[:, :ncall, :], tab_view(tab_tensor, b, nrows_cap),
                        it[:, :(ncall * P) // 16],
                        ncall * P, ncall * P, esize,
                        elem_step=(D if er > 1 else None),
                        single_packet=sp, queue_num=qrr[0] % 4)
                    qrr[0] += 1
                    et = sbE.tile([P, mxcall, P], bf16, tag="eq")
                    if not CONSTEQ:
                        # ACT's HWDGE ring: parallel to sync's (idx/cast/out)
                        nc.scalar.dma_start(et[:, :ncall, :],
                                            eqs[:, s0:s0 + ncall, :])
                    if NOMM:
                        chunk_pos += ncall
                        continue
                    for j in range(ncall):
                        g = chunk_pos + j
                        bb, ww = pl.sched[g]
                        kk = int(pl.K[bb, ww])
                        off = g - pl.run_start[(bb, ww)]
                        if off == 0:
                            open_run[ww] = psR.tile([P, P], f32, tag="run",
                                                    name="runps")
                        ps = open_run[ww]
                        nc.tensor.matmul(ps[:], lhsT=msg[:, j, :D],
                                         rhs=et[:, j, :],
                                         start=(off == 0), stop=(off == kk - 1))
                        if off == kk - 1:
                            aw = acc[:, ww * P:(ww + 1) * P]
                            if ww in drained:
                                nc.vector.tensor_add(aw, aw, ps[:])
                            else:
                                tt_copy(aw, ps[:])
                                drained.add(ww)
                            del open_run[ww]
                            if pl.last_bucket[ww] == bb:
                                if layer == 1:
                                    # table rows must be node-major: transpose
                                    # the [feat, dst] acc window via PE
                                    tps = psM.tile([P, P], f32, tag="tps")
                                    nc.tensor.transpose(out=tps[:], in_=aw,
                                                        identity=ident[:])
                                    xws = sbO.tile([P, P], bf16, tag="xws")
                                    tt_copy(xws[:], tps[:])
                                    nc.sync.dma_start(
                                        ag_in.ap()[ww * P:(ww + 1) * P, :],
                                        xws[:])
                                else:
                                    ops = psM.tile([P, D], f32, tag="ops")
                                    nc.tensor.matmul(ops[:], lhsT=aw,
                                                     rhs=w12t[:],
                                                     start=True, stop=True)
                                    ot = sbO.tile([P, D], f32, tag="ot")
                                    tt_copy(ot[:], ops[:])
                                    nc.sync.dma_start(
                                        out[ww * P:(ww + 1) * P, :], ot[:])
                    chunk_pos += ncall

            edge_pass(tab1_t, NPAD + 2, 1)

            if DEBUG_L1:
                nc.sync.dma_start(dbg_acc[:], acc[:])
                continue

            nc.gpsimd.collective_compute(
                "AllGather", mybir.AluOpType.bypass,
                ins=[ag_in.ap()[:]], outs=[tab2_t.ap()[:]],
                replica_groups=[list(range(NCORES))])

            if DEBUG_L2:
                for bb in range(NB):
                    nc.sync.dma_start(
                        dbg_tab2[bb * BUCKET:(bb + 1) * BUCKET, :],
                        tab2_t.ap()[bb * BUCKET:(bb + 1) * BUCKET, :])
                continue

            # layer-2 drains overwrite acc (copy-first per window); Tile WAR
            # tracking orders them after the layer-1 casts
            edge_pass(tab2_t, NPAD, 2)

    nc.compile()
    return nc


_PROG_CACHE = {}


def run(x, edge_index, t_index, W1, W2, freq, phase=None, b1=None, b2=None,
        nc_prog=None):
    pl, in_maps, gpos, corr = preprocess(x, edge_index, t_index, W1, W2, freq,
                                         phase, b1, b2)
    if nc_prog is not None:
        nc = nc_prog
    else:
        key = (pl.nchunk, len(pl.calls), CFG["elem_rows"], CFG["gcall"])
        if key not in _PROG_CACHE:
            _PROG_CACHE[key] = build_program(pl)
        nc = _PROG_CACHE[key]
    res = run_bass_kernel_spmd(nc, in_maps, list(range(NCORES)))
    full = np.concatenate([res.results[c]["out"] for c in range(NCORES)], axis=0)
    out = full[gpos]
    if corr is not None:
        out = out + corr
    return np.ascontiguousarray(out.astype(np.float32))


def kernel(x, edge_index, t_index, W1, b1, W2, b2, freq, phase):
    x = np.asarray(x, dtype=np.float32)
    t_index = np.asarray(t_index, dtype=np.float32)
    return run(x, np.asarray(edge_index), t_index,
               np.asarray(W1, np.float32), np.asarray(W2, np.float32),
               np.asarray(freq, np.float32),
               phase=np.asarray(phase, np.float32),
               b1=np.asarray(b1, np.float32), b2=np.asarray(b2, np.float32))
